# revision 25
# baseline (speedup 1.0000x reference)
"""Sliding-window GQA attention (T=4096, DIM=2048, H=16, KVH=4, D=128, W=1024)
as an 8-core SPMD Trainium2 Bass/Tile kernel.

Sharding (v7): 4-way sequence x 2-way head. Core c covers queries
[1024*(c%4), +1024) for heads [8*(c//4), +8). The 1024-slot K/V halo is
amortized over 2x the queries per core (halo recompute drops from 2x to 1x
of owned work, -28us PE/core vs 8-way sequence). The two cores sharing a
q-range emit partial outputs (linear in heads); the host sums them.

Phases (attention keeps the same tuned 16-virtual-head x 12-tile structure:
8 local heads x 2 q-blocks of 512):
  A : K^T (RoPE'd, bf16) and V (natural, bf16) over 4 spans of 512 kv slots
  A2: 8 local Q^T head-halves projected + RoPE'd (chains reuse the resident
      own-x span tiles; rope matmuls software-pipelined one unit behind)
  B : attention; S(unit+3) issued before PV(unit); merged full-width exp
      units; valid-vector denominator on left-padding tiles
  C : partial O^T projection, bf16 out
All matmul operands bf16 (PSUM fp32), fat-row DMA layouts, exact spans.
"""

import math
import os
import sys

import numpy as np


def _ensure_paths():
    for p in (
        "/root/.axon_site",
        "/root/.axon_site/_ro/trn_rl_repo",
        "/root/.axon_site/_ro/pypackages",
        "/opt/trn_rl_repo",
        "/opt/pypackages",
    ):
        if os.path.isdir(p) and p not in sys.path:
            sys.path.append(p)


try:
    import concourse.bass as bass  # noqa: F401
except ImportError:
    _ensure_paths()

import ml_dtypes

import concourse.bass as bass
import concourse.mybir as mybir
import concourse.tile as tile
from concourse import bacc
from concourse.bass_utils import run_bass_kernel_spmd

# ---------------------------------------------------------------- constants
N_CORES = 8
SEQ_SH = 4                 # sequence shards
T = 4096
DIM = 2048
H = 16
KVH = 4
D = 128
WIN = 1024
ROPE_BASE = 10000.0

TQ = T // SEQ_SH           # 1024 queries per core
QB = TQ // 512             # 2 q-blocks of 512
TKV = TQ + WIN             # 2048 kv slots per core
NSP = TKV // 512           # 4 kv spans of 512
NMT = 12                   # kv tiles per (head, q-block) window of 1536
NCC = DIM // 128           # 16 contraction chunks
SCALE = 1.0 / math.sqrt(D)
LH = H // (N_CORES // SEQ_SH)   # 8 local heads per core
LKV = LH // 4              # 2 local kv groups

F32 = mybir.dt.float32
BF16 = mybir.dt.bfloat16
BF = ml_dtypes.bfloat16

# per kv-tile m: exact (qlo, qhi) span of q (within its 512 q-block)
SPANS = {
    0: (0, 128), 1: (0, 256), 2: (0, 384), 3: (0, 512),
    4: (0, 512), 5: (0, 512), 6: (0, 512), 7: (0, 512),
    8: (0, 512), 9: (128, 512), 10: (256, 512), 11: (384, 512),
}
MASKS = {
    0: ("maskB", 0, 128), 1: ("maskB", 128, 256),
    2: ("maskB", 256, 384), 3: ("maskB", 384, 512),
    4: None, 5: None, 6: None, 7: None,
    8: ("maskA", 0, 128), 9: ("maskA", 128, 256),
    10: ("maskA", 256, 384), 11: ("maskA", 384, 512),
}
# Pipeline units: one PSUM bank + one exp each, all full 512-wide.
# {4} first (start=True clears the bank), {2,11} last (stop on 11).
UNITS = [(4,), (5,), (6,), (7,), (9, 0), (10, 1), (3,), (8,), (2, 11)]
UOFF = {9: 0, 0: 384, 10: 0, 1: 256, 2: 0, 11: 384}  # col offset in unit bank
MERGED = {0: 0, 1: 1, 2: 2}  # m -> valid-table block (m9/10/11 never padded)
LOOK = 3                   # S-unit lookahead depth in phase B


# ---------------------------------------------------------------- device code
_NC_CACHE = None


def _build():
    global _NC_CACHE
    if _NC_CACHE is not None:
        return _NC_CACHE

    nc = bacc.Bacc("TRN2", target_bir_lowering=False, debug=False,
                   num_devices=N_CORES)

    # DRAM I/O (per-core contents via in_maps). Fat-row packed layouts:
    #   xkvT[s*2048 + c*128 + r, t]  = x^T chunk, span s, slot t
    #   wq[p*128+r, c*256+jc] = Wq[c*128+r, (hh*8+2p)*128 ... ]  (8KB rows)
    #   wk[r, c*256+gc]       = Wk[c*128+r, hh*256+gc]           (8KB rows)
    #   wo[np*128+r, hl*256+jc] = Wo[(hh*8+hl)*128+r, np*256+jc] (4KB rows)
    xkvT = nc.dram_tensor("xkvT", [NSP * DIM, 512], BF16,
                          kind="ExternalInput").ap()
    wq = nc.dram_tensor("wq", [4 * 128, 4096], BF16, kind="ExternalInput").ap()
    wk = nc.dram_tensor("wk", [128, NCC * 256], BF16, kind="ExternalInput").ap()
    wv = nc.dram_tensor("wv", [128, NCC * 256], BF16, kind="ExternalInput").ap()
    wo = nc.dram_tensor("wo", [8 * 128, 2048], BF16, kind="ExternalInput").ap()
    cosq = nc.dram_tensor("cosq", [D, TQ], F32, kind="ExternalInput").ap()
    sinq = nc.dram_tensor("sinq", [D, TQ], F32, kind="ExternalInput").ap()
    cosk = nc.dram_tensor("cosk", [NSP * D, 512], F32, kind="ExternalInput").ap()
    sink = nc.dram_tensor("sink", [NSP * D, 512], F32, kind="ExternalInput").ap()
    kbias = nc.dram_tensor("kbias", [128, QB * NMT], F32,
                           kind="ExternalInput").ap()
    valid = nc.dram_tensor("valid", [128, QB * 384], BF16,
                           kind="ExternalInput").ap()
    maskB = nc.dram_tensor("maskB", [128, 128], BF16, kind="ExternalInput").ap()
    maskA = nc.dram_tensor("maskA", [128, 128], BF16, kind="ExternalInput").ap()
    rotp = nc.dram_tensor("rotp", [128, 128], BF16, kind="ExternalInput").ap()
    ones = nc.dram_tensor("ones", [128, 128], BF16, kind="ExternalInput").ap()
    outT = nc.dram_tensor("outT", [DIM, TQ], BF16, kind="ExternalOutput").ap()

    mask_dram = {"maskB": maskB, "maskA": maskA}

    with tile.TileContext(nc) as tc:
        _emit(nc, tc, xkvT, wq, wk, wv, wo, cosq, sinq, cosk, sink,
              kbias, valid, mask_dram, rotp, ones, outT)

    nc.compile()
    _NC_CACHE = nc
    return nc


def _emit(nc, tc, xkvT, wq, wk, wv, wo, cosq, sinq, cosk, sink,
          kbias, valid, mask_dram, rotp, ones, outT):
    from contextlib import ExitStack

    ctx = ExitStack()
    with ctx:
        # SBUF pools (~190KB/partition)
        consts = ctx.enter_context(tc.tile_pool(name="consts", bufs=1))
        ropet = ctx.enter_context(tc.tile_pool(name="ropet", bufs=4))
        xsp = ctx.enter_context(tc.tile_pool(name="xsp", bufs=3 * NCC))
        wkp = ctx.enter_context(tc.tile_pool(name="wkp", bufs=1))
        wvp = ctx.enter_context(tc.tile_pool(name="wvp", bufs=1))
        wqp = ctx.enter_context(tc.tile_pool(name="wqp", bufs=4))
        wop = ctx.enter_context(tc.tile_pool(name="wop", bufs=3))
        qtp = ctx.enter_context(tc.tile_pool(name="qtp", bufs=LH))
        ktp = ctx.enter_context(tc.tile_pool(name="ktp", bufs=LKV))
        vp = ctx.enter_context(tc.tile_pool(name="vp", bufs=QB * NMT - 8))
        ytp = ctx.enter_context(tc.tile_pool(name="ytp", bufs=LH))
        pp = ctx.enter_context(tc.tile_pool(name="pp", bufs=5))
        tmp = ctx.enter_context(tc.tile_pool(name="tmp", bufs=4))
        t12 = ctx.enter_context(tc.tile_pool(name="t12", bufs=4))
        fin = ctx.enter_context(tc.tile_pool(name="fin", bufs=2))
        # PSUM: exactly 8 banks
        ps_acc = ctx.enter_context(tc.tile_pool(name="ps_acc", bufs=4,
                                                space="PSUM"))
        ps_s = ctx.enter_context(tc.tile_pool(name="ps_s", bufs=3,
                                              space="PSUM"))
        ps_r = ctx.enter_context(tc.tile_pool(name="ps_r", bufs=1,
                                              space="PSUM"))

        Exp = mybir.ActivationFunctionType.Exp
        Copy = mybir.ActivationFunctionType.Copy

        # ---- persistent weights (gpsimd queue); wk split for early start
        wk_sub = {}      # c -> (tile, col_base)
        wk_splits = [(0, 1), (1, 1), (2, 2), (4, 4), (8, 8)]
        for si, (c0, ncs) in enumerate(wk_splits):
            wkt = wkp.tile([128, ncs * 256], BF16, tag=f"wk{si}",
                           name=f"wk_sub{si}", bufs=1)
            nc.gpsimd.dma_start(wkt[:], wk[:, c0 * 256:(c0 + ncs) * 256])
            for c in range(c0, c0 + ncs):
                wk_sub[c] = (wkt, (c - c0) * 256)
        wv_sub = []
        for q4 in range(4):
            wvt = wvp.tile([128, 4 * 256], BF16, tag="wv", name=f"wv_sub{q4}",
                           bufs=4)
            nc.gpsimd.dma_start(wvt[:], wv[:, q4 * 1024:(q4 + 1) * 1024])
            wv_sub.append(wvt)

        def wk_sl(c, g):
            t, base = wk_sub[c]
            return t[:, base + g * 128:base + (g + 1) * 128]

        def wv_sl(c):
            return wv_sub[c // 4][:, (c % 4) * 256:(c % 4 + 1) * 256]

        # ---- phase A: K^T (RoPE'd) and V over NSP spans of 512 kv slots
        kt_sb = [ktp.tile([128, TKV], BF16, tag="kt", name=f"kt{g}")
                 for g in range(LKV)]
        v_sb = [vp.tile([128, 256], BF16, tag="v", name=f"v{mt}",
                        bufs=QB * NMT - 8)
                for mt in range(QB * NMT - 8)]   # 16 slot-tiles x (2g*128)

        consts_loaded = [False]
        const_sb = {}

        def cload(ap, shape, dtype, tag):
            t = consts.tile(shape, dtype, tag=tag, name=tag)
            nc.sync.dma_start(t[:], ap[:])
            return t

        xs_all = []
        for s in range(NSP):
            xs = []
            for c in range(NCC):
                xt = xsp.tile([128, 512], BF16, tag="xs", name=f"xs{s}_{c}")
                nc.sync.dma_start(
                    xt[:], xkvT[s * DIM + c * 128:s * DIM + (c + 1) * 128, :])
                xs.append(xt)
            xs_all.append(xs)
            cosk_s = ropet.tile([128, 512], F32, tag="rt", name=f"cosk{s}")
            nc.sync.dma_start(cosk_s[:], cosk[s * 128:(s + 1) * 128, :])
            sink_s = ropet.tile([128, 512], F32, tag="rt", name=f"sink{s}")
            nc.sync.dma_start(sink_s[:], sink[s * 128:(s + 1) * 128, :])
            if not consts_loaded[0]:
                const_sb["rotp"] = cload(rotp, [128, 128], BF16, "rotp")
                const_sb["ones"] = cload(ones, [128, 128], BF16, "ones")
                const_sb["kbias"] = cload(kbias, [128, QB * NMT], F32, "kbias")
                const_sb["valid"] = cload(valid, [128, QB * 384], BF16,
                                          "valid")
                const_sb["maskB"] = cload(mask_dram["maskB"], [128, 128],
                                          BF16, "maskB")
                const_sb["maskA"] = cload(mask_dram["maskA"], [128, 128],
                                          BF16, "maskA")
                for qb in range(QB):
                    const_sb[f"cosq{qb}"] = cload(
                        cosq[:, qb * 512:(qb + 1) * 512], [128, 512], F32,
                        f"cosq{qb}")
                    const_sb[f"sinq{qb}"] = cload(
                        sinq[:, qb * 512:(qb + 1) * 512], [128, 512], F32,
                        f"sinq{qb}")
                consts_loaded[0] = True

            # K^T projection: LKV chains; second chain's first writes
            # deferred so last span's kps WAR (rope t2-mul) is hidden
            kps = [ps_acc.tile([128, 512], F32, tag="acc", name=f"kps{s}_{g}")
                   for g in range(LKV)]

            def kmm(c, g):
                nc.tensor.matmul(kps[g][:], wk_sl(c, g), xs[c][:],
                                 start=(c == 0), stop=(c == NCC - 1))

            for c in range(4):
                kmm(c, 0)
            for c in range(4):
                kmm(c, 1)
            for c in range(4, NCC):
                for g in range(LKV):
                    kmm(c, g)

            ssb = []
            for g in range(LKV):
                sg = tmp.tile([128, 512], BF16, tag="ssb", name=f"ssb{s}_{g}")
                nc.scalar.activation(sg[:], kps[g][:], Copy)
                ssb.append(sg)

            def ropek(g):
                r_ps = ps_r.tile([128, 512], F32, tag="rp", name=f"rk{s}_{g}")
                nc.tensor.matmul(r_ps[:], const_sb["rotp"][:], ssb[g][:],
                                 start=True, stop=True)
                t1 = t12.tile([128, 512], F32, tag="t12", name=f"kt1_{s}_{g}")
                nc.vector.tensor_mul(t1[:], r_ps[:], sink_s[:])
                t2 = t12.tile([128, 512], F32, tag="t12", name=f"kt2_{s}_{g}")
                nc.vector.tensor_mul(t2[:], kps[g][:], cosk_s[:])
                nc.vector.tensor_add(kt_sb[g][:, s * 512:(s + 1) * 512],
                                     t1[:], t2[:])

            # V projection (natural layout, both groups as 256-wide rhs),
            # 2 passes of 2 banks, rope matmuls interleaved
            vps = {}

            def vchain(tts, c0, c1):
                for c in range(c0, c1):
                    for tt in tts:
                        nc.tensor.matmul(
                            vps[tt][:],
                            xs[c][:, tt * 128:(tt + 1) * 128],
                            wv_sl(c),
                            start=(c == 0), stop=(c == NCC - 1))

            for tt in (0, 1):
                vps[tt] = ps_s.tile([128, 256], F32, tag="sps",
                                    name=f"vps{s}_{tt}")
            vchain((0, 1), 0, 8)
            ropek(0)
            vchain((0, 1), 8, NCC)
            ropek(1)
            for tt in (0, 1):
                nc.scalar.activation(v_sb[4 * s + tt][:], vps[tt][:], Copy)
            for tt in (2, 3):
                vps[tt] = ps_s.tile([128, 256], F32, tag="sps",
                                    name=f"vps{s}_{tt}")
            vchain((2, 3), 0, NCC)
            for tt in (2, 3):
                nc.scalar.activation(v_sb[4 * s + tt][:], vps[tt][:], Copy)

        # ---- phase A2: 8 local Q^T heads projected + RoPE'd. Chains reuse
        # the resident own-x span tiles (spans 2,3 <-> q-blocks 0,1); rope
        # matmuls of a unit are emitted mid-chain of the next unit.
        qts = [qtp.tile([128, TQ], BF16, tag="qt", name=f"qt{hl}")
               for hl in range(LH)]

        def ropeq(p_, qb, j, qpair):
            sg = tmp.tile([128, 512], BF16, tag="ssb", name=f"sq{p_}{qb}{j}")
            nc.scalar.activation(sg[:], qpair[j][:], Copy)
            r_ps = ps_s.tile([128, 512], F32, tag="sps", name=f"rq{p_}{qb}{j}")
            nc.tensor.matmul(r_ps[:], const_sb["rotp"][:], sg[:],
                             start=True, stop=True)
            t1 = t12.tile([128, 512], F32, tag="t12", name=f"qt1_{p_}{qb}{j}")
            nc.vector.tensor_mul(t1[:], r_ps[:], const_sb[f"sinq{qb}"][:])
            t2 = t12.tile([128, 512], F32, tag="t12", name=f"qt2_{p_}{qb}{j}")
            nc.vector.tensor_mul(t2[:], qpair[j][:], const_sb[f"cosq{qb}"][:])
            nc.vector.tensor_add(qts[2 * p_ + j][:, qb * 512:(qb + 1) * 512],
                                 t1[:], t2[:])

        wqts = {}
        prev = None  # (p_, qb, qpair) whose ropes are pending
        for p_ in range(LH // 2):
            wqt = wqp.tile([128, 4096], BF16, tag="wq", name=f"wqt{p_}")
            nc.sync.dma_start(wqt[:], wq[p_ * 128:(p_ + 1) * 128, :])
            wqts[p_] = wqt
        for p_ in range(LH // 2):
            wqt = wqts[p_]
            for qb in range(QB):
                qpair = [ps_acc.tile([128, 512], F32, tag="acc",
                                     name=f"qps{p_}{qb}{j}")
                         for j in range(2)]
                xsq = xs_all[2 + qb]
                for c in range(NCC):
                    if c == 6 and prev is not None:
                        ropeq(prev[0], prev[1], 0, prev[2])
                    if c == 10 and prev is not None:
                        ropeq(prev[0], prev[1], 1, prev[2])
                        prev = None
                    for j in range(2):
                        nc.tensor.matmul(qpair[j][:],
                                         wqt[:, c * 256 + j * 128:
                                             c * 256 + (j + 1) * 128],
                                         xsq[c][:],
                                         start=(c == 0), stop=(c == NCC - 1))
                prev = (p_, qb, qpair)
        ropeq(prev[0], prev[1], 0, prev[2])
        ropeq(prev[0], prev[1], 1, prev[2])

        # ---- phase B: attention over 16 virtual heads (hl, qb)
        yt_sb = [ytp.tile([128, TQ], BF16, tag="yt", name=f"yt{hl}")
                 for hl in range(LH)]

        ucount = [0]
        for hl in range(LH):
            for qb in range(QB):
                gl = hl // 4
                vh = f"{hl}_{qb}"
                acc_y = ps_acc.tile([128, 512], F32, tag="acc",
                                    name=f"yps{vh}")
                acc_d = ps_acc.tile([128, 512], F32, tag="acc",
                                    name=f"dps{vh}")
                p_l = {}

                def qk(ui, gl=gl, qb=qb, vh=vh, hl=hl, p_l=p_l):
                    unit = UNITS[ui]
                    merged = len(unit) > 1
                    u = ucount[0]
                    ucount[0] += 1
                    pool = ps_r if u % 4 == 3 else ps_s
                    tagn = "rp" if u % 4 == 3 else "sps"
                    sps = pool.tile([128, 512], F32, tag=tagn,
                                    name=f"sps{vh}_{unit[0]}")
                    ext = 0
                    for m in unit:
                        qlo, qhi = SPANS[m]
                        w = qhi - qlo
                        off = UOFF[m] if merged else 0
                        nc.tensor.matmul(
                            sps[:, off:off + w],
                            kt_sb[gl][:, qb * 512 + m * 128:
                                      qb * 512 + (m + 1) * 128],
                            qts[hl][:, qb * 512 + qlo:qb * 512 + qhi],
                            start=True, stop=True)
                        ext = max(ext, off + w)
                    p = pp.tile([128, 512], BF16, tag="p",
                                name=f"p{vh}_{unit[0]}")
                    bias = 0.0 if merged else \
                        const_sb["kbias"][:, qb * NMT + unit[0]:
                                          qb * NMT + unit[0] + 1]
                    nc.scalar.activation(p[:, :ext], sps[:, :ext], Exp,
                                         bias=bias, scale=SCALE)
                    for m in unit:
                        mk = MASKS[m]
                        if mk is not None:
                            qlo, qhi = SPANS[m]
                            off = (UOFF[m] if merged else 0) - qlo
                            name_, lo, hi = mk
                            nc.vector.tensor_mul(p[:, lo + off:hi + off],
                                                 p[:, lo + off:hi + off],
                                                 const_sb[name_][:])
                    p_l[ui] = p

                def pv(ui, gl=gl, qb=qb, acc_y=acc_y, acc_d=acc_d, p_l=p_l):
                    unit = UNITS[ui]
                    merged = len(unit) > 1
                    p = p_l.pop(ui)
                    first = ui == 0
                    last_unit = ui == len(UNITS) - 1
                    for m in unit:
                        qlo, qhi = SPANS[m]
                        w = qhi - qlo
                        off = UOFF[m] if merged else 0
                        last = last_unit and m == unit[-1]
                        if m in MERGED:
                            vi = qb * 384 + MERGED[m] * 128
                            den_st = const_sb["valid"][:, vi:vi + 128]
                        else:
                            den_st = const_sb["ones"][:]
                        nc.tensor.matmul(
                            acc_y[:, qlo:qhi],
                            v_sb[4 * qb + m][:, gl * 128:(gl + 1) * 128],
                            p[:, off:off + w], start=first, stop=last)
                        nc.tensor.matmul(acc_d[:, qlo:qhi], den_st,
                                         p[:, off:off + w], start=first,
                                         stop=last)
                        first = False

                for i in range(LOOK):
                    qk(i)
                for i in range(len(UNITS)):
                    if i + LOOK < len(UNITS):
                        qk(i + LOOK)
                    pv(i)

                rcp = fin.tile([128, 512], F32, tag="rcp", name=f"rcp{vh}")
                nc.vector.reciprocal_approx_fast(rcp[:], acc_d[:])
                nc.vector.tensor_mul(
                    yt_sb[hl][:, qb * 512:(qb + 1) * 512], acc_y[:], rcp[:])

        # ---- phase C: partial O^T projection (local heads only; host sums
        # the two partials per q-range)
        for n0 in range(0, NCC, 2):
            np_ = n0 // 2
            wot = wop.tile([128, 2048], BF16, tag="wo", name=f"wot{np_}")
            nc.sync.dma_start(wot[:], wo[np_ * 128:(np_ + 1) * 128, :])
            for qb in range(QB):
                opair = [ps_acc.tile([128, 512], F32, tag="acc",
                                     name=f"ops{n0}_{qb}_{j}")
                         for j in range(2)]
                for hl in range(LH):
                    for j in range(2):
                        nc.tensor.matmul(
                            opair[j][:],
                            wot[:, hl * 256 + j * 128:hl * 256 + (j + 1) * 128],
                            yt_sb[hl][:, qb * 512:(qb + 1) * 512],
                            start=(hl == 0), stop=(hl == LH - 1))
                for j in range(2):
                    osb = fin.tile([128, 512], BF16, tag="osb",
                                   name=f"osb{n0}_{qb}_{j}")
                    nc.scalar.activation(osb[:], opair[j][:], Copy)
                    nc.sync.dma_start(
                        outT[(n0 + j) * 128:(n0 + j + 1) * 128,
                             qb * 512:(qb + 1) * 512], osb[:])


# ---------------------------------------------------------------- host side
def _host_inputs(x, Wq, Wk, Wv, Wo):
    x = np.asarray(x, dtype=np.float32).reshape(T, DIM)
    Wq = np.asarray(Wq, np.float32)
    Wk = np.asarray(Wk, np.float32)
    Wv = np.asarray(Wv, np.float32)
    Wo = np.asarray(Wo, np.float32)

    inv_freq = 1.0 / (ROPE_BASE ** (np.arange(0, D, 2, dtype=np.float64) / D))
    dfreq = np.concatenate([inv_freq, inv_freq])  # [128] per-dim freq

    u = np.arange(128)[:, None]
    maskB = (np.arange(128)[None, :] < u).astype(BF)        # qq>=u -> 0
    maskA = (u <= np.arange(128)[None, :]).astype(BF)       # u>qq -> 0

    rotp = np.zeros((128, 128), np.float32)
    d = np.arange(128)
    rotp[(d + 64) % 128, d] = 1.0  # out[d] = in[(d+64)%128]
    rotp = rotp.astype(BF)

    ones = np.ones((128, 128), BF)

    # per head-half hh: packed weight slices
    wq_hh, wk_hh, wv_hh, wo_hh = [], [], [], []
    for hh in range(2):
        wq_hh.append(np.ascontiguousarray(
            Wq[:, hh * 1024:(hh + 1) * 1024]
            .reshape(NCC, 128, 4, 256).transpose(2, 1, 0, 3)
            .reshape(4 * 128, 4096)).astype(BF))
        wk_hh.append(np.ascontiguousarray(
            Wk[:, hh * 256:(hh + 1) * 256]
            .reshape(NCC, 128, 256).transpose(1, 0, 2)
            .reshape(128, NCC * 256)).astype(BF))
        wv_hh.append(np.ascontiguousarray(
            Wv[:, hh * 256:(hh + 1) * 256]
            .reshape(NCC, 128, 256).transpose(1, 0, 2)
            .reshape(128, NCC * 256)).astype(BF))
        wo_hh.append(np.ascontiguousarray(
            Wo[hh * 1024:(hh + 1) * 1024, :]
            .reshape(8, 128, 8, 256).transpose(2, 1, 0, 3)
            .reshape(8 * 128, 2048)).astype(BF))

    in_maps = []
    for c in range(N_CORES):
        rc = c % SEQ_SH
        hh = c // SEQ_SH
        qs = rc * TQ
        xkv = np.zeros((TKV, DIM), np.float32)  # [2048, 2048]
        lo = qs - WIN
        src_lo = max(0, lo)
        xkv[src_lo - lo:TKV] = x[src_lo:qs + TQ]

        pos_q = np.arange(qs, qs + TQ, dtype=np.float64)
        pos_k = np.arange(lo, qs + TQ, dtype=np.float64)
        angq = dfreq[:, None] * pos_q[None, :]  # [128, 1024]
        angk = dfreq[:, None] * pos_k[None, :]  # [128, 2048]
        sgn = np.where(np.arange(D) < D // 2, -1.0, 1.0)[:, None]

        kb = np.zeros((128, QB * NMT), np.float32)
        vld = np.zeros((128, QB * 384), np.float32)
        for qb in range(QB):
            winq = WIN - qs - 512 * qb   # local slots below this are padding
            for m in range(NMT):
                t_loc = 128 * m + np.arange(128)
                kb[:, qb * NMT + m] = np.where(t_loc < winq, -30.0, 0.0)
            for vi, m in enumerate((0, 1, 2)):
                t_loc = 128 * m + np.arange(128)
                vld[:, qb * 384 + vi * 128:qb * 384 + (vi + 1) * 128] = \
                    np.where(t_loc < winq, 0.0, 1.0)[:, None]

        in_maps.append({
            "xkvT": np.ascontiguousarray(
                xkv.T.reshape(DIM, NSP, 512).transpose(1, 0, 2)
                .reshape(NSP * DIM, 512)).astype(BF),
            "wq": wq_hh[hh], "wk": wk_hh[hh], "wv": wv_hh[hh],
            "wo": wo_hh[hh],
            "cosq": np.cos(angq).astype(np.float32),
            "sinq": (sgn * np.sin(angq)).astype(np.float32),
            "cosk": np.ascontiguousarray(np.cos(angk).astype(np.float32)
                .reshape(D, NSP, 512).transpose(1, 0, 2))
                .reshape(NSP * D, 512),
            "sink": np.ascontiguousarray(((sgn * np.sin(angk))
                .astype(np.float32))
                .reshape(D, NSP, 512).transpose(1, 0, 2))
                .reshape(NSP * D, 512),
            "kbias": kb,
            "valid": vld.astype(BF),
            "maskB": maskB, "maskA": maskA,
            "rotp": rotp,
            "ones": ones,
        })
    return in_maps


def kernel(x, Wq, Wk, Wv, Wo, _trace=False, _trace_kwargs=None):
    nc = _build()
    in_maps = _host_inputs(x, Wq, Wk, Wv, Wo)
    res = run_bass_kernel_spmd(nc, in_maps, core_ids=list(range(N_CORES)),
                               trace=_trace, **(_trace_kwargs or {}))
    out = np.zeros((1, T, DIM), np.float32)
    for c in range(N_CORES):
        rc = c % SEQ_SH
        out[0, rc * TQ:(rc + 1) * TQ, :] += \
            res.results[c]["outT"].T.astype(np.float32)
    if _trace:
        kernel.last_results = res
    return out
